# revision 1
# baseline (speedup 1.0000x reference)
"""Batchelor gpuNUFFT-adjoint (bilinear gridding + IFFT + deapod + coil
combine + motion warp + temporal sum) on 8 Trainium2 NeuronCores.

Sharding: one motion state (frame) per core. Inside each core:
  - density-compensated bilinear gridding onto a 2x oversampled 640x640
    grid, accumulated DIRECTLY IN SBUF (bf16) via SBUF-destination
    dma_scatter_add (tokens_per_rank=128, one grid row per descriptor;
    the slot parity bit routes to S0M/S1M ("own") vs S0T ("other")
    tiles, see UNITS below)
  - IFFT+fftshift+crop+deapodization as two DFT matmul passes off the
    SBUF-resident grid (host-precomputed DFT matrices, bf16); pass 1
    for the 60% segment overlaps the remaining scatter stream, and
    pass1-late(c+1) is emitted before pass2(c) so psum->SBUF copies
    hide under matmuls
  - conjugate coil combine with the sensitivity maps (DVE)
Temporal sum + bilinear motion warp of the 8 per-core frames on host.

Conflict handling: descriptors hitting the same grid token within one
scatter call race (verified on HW), so duplicates are split into rounds
(one call per (bin, round, CAP-chunk)); bins are (u-chunk, j-segment)
with j split so token ids fit int16. Padding descriptors add +0 to
cells the frame never touches (+0 races are harmless).
"""
import sys

sys.path.insert(0, "/opt/trn_rl_repo")
sys.path.insert(0, "/opt/trn_rl_repo/concourse")

import numpy as np
from contextlib import ExitStack

import concourse.bass as bass
import concourse.tile as tile
from concourse import bacc, mybir
from concourse.bass_utils import run_bass_kernel_spmd

F32 = mybir.dt.float32
BF16 = mybir.dt.bfloat16
I16 = mybir.dt.int16

import os as _os

NX, NC, NT, M, OS = 320, 8, 8, 65536, 2
G = OS * NX                  # 640
UC = 5                       # u chunks of 128 rows
CAP = int(_os.environ.get("K_CAP", "4096"))   # descriptors per scatter call
NSWQ = int(_os.environ.get("K_NSWQ", "1"))    # SWDGE queues
SCRATCH = int(_os.environ.get("K_SCRATCH", "16384"))
YC = [0, 128, 256]
YCN = [128, 128, 64]
# Grid tiles per u-chunk (junit g in [0,160), A = j<320, B = j>=320):
#   S0M: [A g0-64   | B g0-64  ] 128 groups (seg0 scatter "own" target)
#   S0T: [A g64-96  | B g64-96 ]  64 groups + 64 waste ("other" target;
#        shape must equal S0M's, upper half never touched)
#   S1M: [A g96-160 | B g96-160] 128 groups (seg1; other == own)
# pass-1 psum units: unit -> (tile_kind, group_off); 64 groups = 128 j
UNITS = [
    ("S0M", 0),    # U0: A g[0,64)    j[0,128)
    ("S0M", 64),   # U1: B g[0,64)    j[320,448)
    ("S0T", 0),    # U2: A/B g[64,96) j[128,192)+[448,512)
    ("S1M", 0),    # U3: A g[96,160)  j[192,320)
    ("S1M", 64),   # U4: B g[96,160)  j[512,640)
]
# W2 row lists per unit (j indices, in o1 partition order)
UNIT_J = [
    list(range(0, 128)),
    list(range(320, 448)),
    list(range(128, 192)) + list(range(448, 512)),
    list(range(192, 320)),
    list(range(512, 640)),
]
EARLY = (0, 1, 2)            # units ready after seg0 scatter
LATE = (3, 4)


def _dft_matrix():
    """W[u, x]: im[x,y] = sum_uv grid[u,v] W[u,x] W[v,y] (shift+crop+deapod
    folded)."""
    xc = np.arange(NX)
    u = np.arange(G)
    xs = (xc - NX // 2) / G
    dapo = np.sinc(xs) ** 2
    ph = np.exp(2j * np.pi * np.outer(u - 320, xc + 480) / G)
    W = ph / G / dapo[None, :]
    return W.astype(np.complex64)


def _plan_frame(traj_t, dcf_t):
    """Descriptor list for one frame: per descriptor (one grid row x one
    junit of one sample): bin, round, token, w0, w1, sample."""
    tx = traj_t[:, 0].astype(np.float32)
    ty = traj_t[:, 1].astype(np.float32)
    d = dcf_t.astype(np.float32)
    u = (tx + np.float32(0.5)) * np.float32(G)
    v = (ty + np.float32(0.5)) * np.float32(G)
    u0 = np.floor(u)
    v0 = np.floor(v)
    du = (u - u0).astype(np.float32)
    dv = (v - v0).astype(np.float32)
    i0 = u0.astype(np.int64) % G
    i1 = (i0 + 1) % G
    j0 = v0.astype(np.int64) % G
    j1 = (j0 + 1) % G
    wr0 = np.float32(1.0) - du
    wr1 = du
    wc0 = (np.float32(1.0) - dv) * d
    wc1 = dv * d

    even = (j0 % 2) == 0
    sE = np.nonzero(even)[0]
    sO = np.nonzero(~even)[0]
    zO = np.zeros(len(sO), np.float32)

    rows, jus, w0s, w1s, smps = [], [], [], [], []
    for (r_, wr_) in ((i0, wr0), (i1, wr1)):
        rows += [r_[sE], r_[sO], r_[sO]]
        jus += [j0[sE] // 2, j0[sO] // 2, j1[sO] // 2]
        w0s += [wc0[sE] * wr_[sE], zO, wc1[sO] * wr_[sO]]
        w1s += [wc1[sE] * wr_[sE], wc0[sO] * wr_[sO], zO]
        smps += [sE, sO, sO]
    r = np.concatenate(rows)
    ju = np.concatenate(jus)
    w0 = np.concatenate(w0s)
    w1 = np.concatenate(w1s)
    smp = np.concatenate(smps)

    ucc = r // 128
    par = (ju >= 160).astype(np.int64)   # 0 = A (j<320), 1 = B
    g = ju - 160 * par
    seg = (g >= 96).astype(np.int64)
    p = r % 128
    # seg0: g<64 -> own (S0M) slot 2*(par*64+g); g in [64,96) -> other
    # (S0T) slot 2*(par*32+g-64)+1. seg1: own (S1M) slot 2*(par*64+g-96).
    in_t = (seg == 0) & (g >= 64)
    u = np.where(seg == 1, par * 64 + g - 96,
                 np.where(in_t, par * 32 + g - 64, par * 64 + g))
    tok = (2 * u + in_t.astype(np.int64)) * 128 + p
    binid = ucc * 2 + seg

    key = binid * 32768 + tok
    order = np.argsort(key, kind="stable")
    ks = key[order]
    n = len(ks)
    newgrp = np.ones(n, bool)
    newgrp[1:] = ks[1:] != ks[:-1]
    gstart = np.maximum.accumulate(np.where(newgrp, np.arange(n), 0))
    rnd = np.empty(n, np.int64)
    rnd[order] = np.arange(n) - gstart
    assert rnd.max() < 64, "conflict-round overflow (pathological traj)"

    # per-bin pad pool: real token values this frame never touches
    # (pads scatter +0 into them; +0 races are harmless)
    own_c = (2 * np.repeat(np.arange(128), 128)) * 128 + np.tile(
        np.arange(128), 128)
    oth_c = (2 * np.repeat(np.arange(64), 128) + 1) * 128 + np.tile(
        np.arange(128), 64)
    cand0 = np.concatenate([own_c, oth_c])
    pads = {}
    for b in range(10):
        cand = cand0 if b % 2 == 0 else own_c
        used = np.zeros(32768, bool)
        used[tok[binid == b]] = True
        pads[b] = cand[~used[cand]]
        assert len(pads[b]) > 0, "no free pad cells (pathological traj)"

    return dict(tok=tok, bin=binid, rnd=rnd, w0=w0, w1=w1, smp=smp, pads=pads)


def _build_schedule(plans):
    """Common (SPMD) call schedule: entries (bin, rnd, call, size).

    Order: per segment, round-major across bins (consecutive same-bin
    calls are >= 5 apart, hiding each call's DMA latency behind other
    bins); segments merged 2:1 favoring seg0 so pass-1 on seg0 columns
    starts well before the scatter stream ends."""
    maxcnt = {}
    for pl in plans:
        key = pl["bin"] * 64 + pl["rnd"]
        uk, cnt = np.unique(key, return_counts=True)
        for k, c in zip(uk.tolist(), cnt.tolist()):
            maxcnt[k] = max(maxcnt.get(k, 0), c)

    seglist = {0: [], 1: []}
    for k in sorted(maxcnt):
        binid, rnd = k // 64, k % 64
        n = maxcnt[k]
        for c in range(-(-n // CAP)):
            sz = min(CAP, n - c * CAP)
            sz = -(-sz // 128) * 128
            seglist[binid % 2].append((binid, rnd, c, sz))
    for s in (0, 1):
        seglist[s].sort(key=lambda e: (e[1], e[2], e[0]))

    # weave a few seg1 calls ahead of seg0's tail so seg1's scatter
    # finishes just as pass-1-early runs out of seg0 work
    s0, s1 = seglist[0], seglist[1]
    k = int(_os.environ.get("K_WEAVE", "4"))
    k = min(k, len(s0), len(s1))
    if k:
        return s0[:-k] + s1[:k] + s0[-k:] + s1[k:]
    return s0 + s1


def _pack_frame(pl, entries, ks16):
    """Pack one frame's descriptors into the common schedule.

    Returns idx [128, TOT16] i16 (replicated x8) and stream
    [128, TOTR, 18] f32 (kspace16 ++ (w0, w1) per descriptor)."""
    import ml_dtypes
    TOT16 = sum(e[3] // 16 for e in entries)
    TOTR = sum(e[3] // 128 for e in entries)
    idx16 = np.empty((16, TOT16), np.int16)
    stream = np.zeros((128, TOTR, 18), ml_dtypes.bfloat16)
    dat = stream[:, :, :16]
    w2 = stream[:, :, 16:18]

    # prefill pads + entry offset tables
    ent_of = {}
    off16 = np.empty(len(entries), np.int64)
    offR = np.empty(len(entries), np.int64)
    o16 = oR = 0
    for ei, (binid, rnd, c, sz) in enumerate(entries):
        ent_of[(binid, rnd, c)] = ei
        off16[ei] = o16
        offR[ei] = oR
        ppool = pl["pads"][binid]
        pad_tok = ppool[np.arange(sz) % len(ppool)]
        idx16[:, o16:o16 + sz // 16] = (
            pad_tok.reshape(sz // 16, 16).T.astype(np.int16))
        o16 += sz // 16
        oR += sz // 128

    # slot within (bin, rnd) group
    key = pl["bin"] * 64 + pl["rnd"]
    order = np.argsort(key, kind="stable")
    ks = key[order]
    n = len(ks)
    newgrp = np.ones(n, bool)
    newgrp[1:] = ks[1:] != ks[:-1]
    gstart = np.maximum.accumulate(np.where(newgrp, np.arange(n), 0))
    slot_sorted = np.arange(n) - gstart

    call_sorted = slot_sorted // CAP
    pos_sorted = slot_sorted % CAP
    ei_sorted = np.array(
        [ent_of[(int(b) // 64, int(b) % 64, int(cc))]
         for b, cc in zip(ks, call_sorted)], np.int64) \
        if False else None
    # vectorized entry lookup: build dense map (bin*64+rnd, call) -> ei
    maxkey = int(ks.max()) + 1
    maxcall = int(call_sorted.max()) + 1
    dense = np.full((maxkey, maxcall), -1, np.int64)
    for (binid, rnd, c), ei in ent_of.items():
        k = binid * 64 + rnd
        if k < maxkey and c < maxcall:
            dense[k, c] = ei
    ei_sorted = dense[ks, call_sorted]
    assert (ei_sorted >= 0).all()

    tok_sorted = pl["tok"][order]
    smp_sorted = pl["smp"][order]
    w0_sorted = pl["w0"][order]
    w1_sorted = pl["w1"][order]

    i16col = off16[ei_sorted] + pos_sorted // 16
    i16row = pos_sorted % 16
    idx16[i16row, i16col] = tok_sorted.astype(np.int16)

    rrow = pos_sorted % 128
    rcol = offR[ei_sorted] + pos_sorted // 128
    dat[rrow, rcol] = ks16[smp_sorted]
    w2[rrow, rcol, 0] = w0_sorted
    w2[rrow, rcol, 1] = w1_sorted
    return np.tile(idx16, (8, 1)), stream


RB = int(_os.environ.get("K_RB", "96"))          # stream-batch rows (descs / 128)
NW = 30           # W blocks in the packed wall tensor: 15 w1 + 15 w2


def _make_batches(entries):
    """Group consecutive entries into DMA batches of <= RB rows.

    """
    batches = []
    cur = []
    rows = 0
    for e in entries:
        r = e[3] // 128
        if cur and rows + r > RB:
            batches.append(cur)
            cur = []
            rows = 0
        cur.append(e)
        rows += r
    if cur:
        batches.append(cur)
    return batches


def _build_program(entries):
    nc = bacc.Bacc("TRN2", target_bir_lowering=False, debug=False,
                   num_swdge_queues=NSWQ, dynamic_dma_scratch_size=SCRATCH)
    TOT16 = sum(e[3] // 16 for e in entries)
    TOTR = sum(e[3] // 128 for e in entries)

    idx_t = nc.dram_tensor("idx", [128, TOT16], I16, kind="ExternalInput")
    str_t = nc.dram_tensor("stream", [128, TOTR, 18], BF16, kind="ExternalInput")
    wall_t = nc.dram_tensor("wall", [128, NW, NX], BF16, kind="ExternalInput")
    csm_t = nc.dram_tensor("csmP", [NC, 128, 3, 2, NX], BF16,
                           kind="ExternalInput")
    out_t = nc.dram_tensor("imT", [2, NX, NX], F32, kind="ExternalOutput")

    with tile.TileContext(nc) as tc, ExitStack() as ctx:
        pool = ctx.enter_context(tc.tile_pool(name="main", bufs=1))
        dbuf = ctx.enter_context(tc.tile_pool(name="dbuf", bufs=2))
        valp = ctx.enter_context(tc.tile_pool(name="valp", bufs=4))
        o1p = ctx.enter_context(tc.tile_pool(name="o1p", bufs=2))
        csmb = ctx.enter_context(tc.tile_pool(name="csmb", bufs=2))
        psum = ctx.enter_context(
            tc.tile_pool(name="psum", bufs=1, space="PSUM"))

        # ---- grid tiles + memsets (all on Act, in first-use order) ----
        gt = {}
        for kind in ("S0M", "S0T", "S1M"):
            for uc in range(UC):
                t_ = pool.tile([128, 128 * 32], BF16, tag=f"g{kind}{uc}",
                               name=f"g{kind}{uc}")
                gt[(kind, uc)] = t_
        # first scatter call's tiles cleared on DVE (idle at t=0) so it
        # isn't gated on the Act memset queue
        nc.vector.memset(gt[("S0M", 0)][:], 0.0)
        nc.vector.memset(gt[("S0T", 0)][:, :64 * 32], 0.0)
        for uc in range(1, UC):
            nc.scalar.memzero(gt[("S0M", uc)][:])
            nc.scalar.memzero(gt[("S0T", uc)][:, :64 * 32])
        for uc in range(UC):
            nc.scalar.memzero(gt[("S1M", uc)][:])

        # ---- acc tiles (memset on Act; first read is late) ----
        acc = {}
        for yc in range(3):
            for comp in ("re", "im"):
                t_ = pool.tile([128, NX], F32, tag=f"acc_{yc}_{comp}")
                nc.scalar.memzero(t_[:])
                acc[(yc, comp)] = t_

        # ---- DFT matrices: single packed load, sliced views ----
        wall = pool.tile([128, NW, NX], BF16, tag="wall")
        nc.sync.dma_start(wall[:], wall_t.ap())
        w1 = {}
        for ci, comp in enumerate(("re", "im", "imn")):
            for uc in range(UC):
                w1[(comp, uc)] = wall[:, ci * UC + uc]
        w2blk = {}
        for ci, comp in enumerate(("re", "im", "imn")):
            for k in range(5):
                w2blk[(comp, k)] = 15 + ci * 5 + k

        # ---- scatter stream (batched loads) ----
        batches = _make_batches(entries)
        o16 = oR = 0
        scall = 0
        for batch in batches:
            brows = sum(e[3] // 128 for e in batch)
            b16 = sum(e[3] // 16 for e in batch)
            st_ = dbuf.tile([128, RB, 18], BF16, tag="stream")
            it_ = dbuf.tile([128, RB * 8], I16, tag="idx")
            nc.sync.dma_start(st_[:, :brows], str_t.ap()[:, oR:oR + brows])
            nc.sync.dma_start(it_[:, :b16], idx_t.ap()[:, o16:o16 + b16])
            r0 = s0 = 0
            for (binid, rnd, c, sz) in batch:
                uc, seg = binid // 2, binid % 2
                R = sz // 128
                vt_ = valp.tile([128, CAP // 128, 2, 16], BF16, tag="val")
                nc.vector.tensor_tensor(
                    out=vt_[:, :R],
                    in0=st_[:, r0:r0 + R, :16].unsqueeze(2)
                    .broadcast_to([128, R, 2, 16]),
                    in1=st_[:, r0:r0 + R, 16:18].unsqueeze(3)
                    .broadcast_to([128, R, 2, 16]),
                    op=mybir.AluOpType.mult)
                own = gt[("S0M" if seg == 0 else "S1M", uc)]
                other = gt[("S0T", uc)] if seg == 0 else own
                nc.gpsimd.dma_scatter_add(
                    out_ap=own[:],
                    in_ap=vt_[:, :R].rearrange("p r a c -> p r (a c)"),
                    idxs_ap=it_[:, s0:s0 + sz // 16],
                    num_idxs=sz,
                    num_idxs_reg=sz,
                    elem_size=32,
                    sbuf_tokens_per_rank=128,
                    parity_reg=0,
                    out_ap_other=other[:],
                    queue_num=scall % NSWQ)
                scall += 1
                r0 += R
                s0 += sz // 16
            o16 += b16
            oR += brows

        # ---- pass 1 ----
        def pass1_unit(k, coil, o1re, o1im, alt):
            ps_re = psum.tile([128, NX], F32, tag=f"p1re{alt}", space="PSUM",
                              name=f"p1re_{coil}_{k}")
            ps_im = psum.tile([128, NX], F32, tag=f"p1im{alt}", space="PSUM",
                              name=f"p1im_{coil}_{k}")
            kind, g0 = UNITS[k]
            for uc in range(UC):
                st = (uc == 0)
                sp = (uc == UC - 1)
                T = gt[(kind, uc)]
                base = g0 * 32 + coil * 2
                tre = T[:, base:base + 127 * 16 + 1:16]
                tim = T[:, base + 1:base + 127 * 16 + 2:16]
                nc.tensor.matmul(ps_re[:], tre, w1[("re", uc)],
                                 start=st, stop=False)
                nc.tensor.matmul(ps_re[:], tim, w1[("imn", uc)],
                                 start=False, stop=sp)
                nc.tensor.matmul(ps_im[:], tre, w1[("im", uc)],
                                 start=st, stop=False)
                nc.tensor.matmul(ps_im[:], tim, w1[("re", uc)],
                                 start=False, stop=sp)
            nc.scalar.copy(o1re[:], ps_re[:])
            nc.scalar.copy(o1im[:], ps_im[:])

        # early units (after seg0 scatter), resident o1 for all coils
        o1r = {}
        alt = 0
        for k in EARLY:
            for coil in range(NC):
                tre = pool.tile([128, NX], BF16, tag=f"o1r_{coil}_{k}_re")
                tim = pool.tile([128, NX], BF16, tag=f"o1r_{coil}_{k}_im")
                pass1_unit(k, coil, tre, tim, alt % 2)
                o1r[(coil, k, "re")] = tre
                o1r[(coil, k, "im")] = tim
                alt += 1

        # late units + pass 2 + combine, software-pipelined per coil:
        # pass1-late(c+1) is emitted before pass2(c) so the Act-engine
        # psum->o1 copies of coil c+1 overlap coil c's pass-2 matmuls.
        def p1late(coil):
            nonlocal alt
            o1c = {}
            for k in EARLY:
                o1c[(k, "re")] = o1r[(coil, k, "re")]
                o1c[(k, "im")] = o1r[(coil, k, "im")]
            for k in LATE:
                tre = o1p.tile([128, NX], BF16, tag=f"o1t_{k}_re",
                               name=f"o1t_{coil}_{k}_re")
                tim = o1p.tile([128, NX], BF16, tag=f"o1t_{k}_im",
                               name=f"o1t_{coil}_{k}_im")
                pass1_unit(k, coil, tre, tim, alt % 2)
                o1c[(k, "re")] = tre
                o1c[(k, "im")] = tim
                alt += 1
            return o1c

        p2alt = 0

        def p2combine(coil, o1c):
            nonlocal p2alt
            ct_ = csmb.tile([128, 3, 2, NX], BF16, tag="csm",
                            name=f"csm_{coil}")
            nc.sync.dma_start(ct_[:], csm_t.ap()[coil])
            for yc in range(3):
                yn = YCN[yc]
                p_re = psum.tile([128, NX], F32, tag=f"p2re{p2alt % 2}",
                                 space="PSUM", name=f"p2re_{coil}_{yc}")
                p_im = psum.tile([128, NX], F32, tag=f"p2im{p2alt % 2}",
                                 space="PSUM", name=f"p2im_{coil}_{yc}")
                p2alt += 1
                for ki, k in enumerate((0, 1, 2, 3, 4)):
                    lre = wall[:, w2blk[("re", k)], YC[yc]:YC[yc] + yn]
                    lim = wall[:, w2blk[("im", k)], YC[yc]:YC[yc] + yn]
                    limn = wall[:, w2blk[("imn", k)], YC[yc]:YC[yc] + yn]
                    ore = o1c[(k, "re")][:]
                    oim = o1c[(k, "im")][:]
                    st = (ki == 0)
                    sp = (ki == 4)
                    nc.tensor.matmul(p_re[:yn], lre, ore, start=st, stop=False)
                    nc.tensor.matmul(p_re[:yn], limn, oim, start=False, stop=sp)
                    nc.tensor.matmul(p_im[:yn], lim, ore, start=st, stop=False)
                    nc.tensor.matmul(p_im[:yn], lre, oim, start=False, stop=sp)
                cr = ct_[:, yc, 0]
                ci = ct_[:, yc, 1]
                junk = pool.tile([128, NX], F32, tag="junk")
                a_re = acc[(yc, "re")]
                a_im = acc[(yc, "im")]
                nc.vector.tensor_tensor(out=junk[:yn], in0=p_re[:yn],
                                        in1=cr[:yn], op=mybir.AluOpType.mult)
                nc.vector.tensor_add(a_re[:yn], a_re[:yn], junk[:yn])
                nc.vector.tensor_tensor(out=junk[:yn], in0=p_im[:yn],
                                        in1=ci[:yn], op=mybir.AluOpType.mult)
                nc.vector.tensor_add(a_re[:yn], a_re[:yn], junk[:yn])
                nc.vector.tensor_tensor(out=junk[:yn], in0=p_im[:yn],
                                        in1=cr[:yn], op=mybir.AluOpType.mult)
                nc.vector.tensor_add(a_im[:yn], a_im[:yn], junk[:yn])
                nc.vector.tensor_tensor(out=junk[:yn], in0=p_re[:yn],
                                        in1=ci[:yn], op=mybir.AluOpType.mult)
                nc.vector.tensor_sub(a_im[:yn], a_im[:yn], junk[:yn])
                if coil == NC - 1:
                    # overlap output DMA of this y-chunk with the next
                    # chunk's pass-2
                    for kk, comp in enumerate(("re", "im")):
                        nc.sync.dma_start(
                            out_t.ap()[kk, YC[yc]:YC[yc] + yn],
                            acc[(yc, comp)][:yn])

        o1c_prev = p1late(0)
        for coil in range(1, NC):
            o1c_next = p1late(coil)
            p2combine(coil - 1, o1c_prev)
            o1c_prev = o1c_next
        p2combine(NC - 1, o1c_prev)

    nc.compile()
    return nc


_PROGRAM_CACHE = {}


def kernel(**inputs):
    traj = np.asarray(inputs["traj"], np.float32)
    dcf = np.asarray(inputs["dcf"], np.float32)
    kspace_r = np.asarray(inputs["kspace_r"], np.float32)
    kspace_i = np.asarray(inputs["kspace_i"], np.float32)
    csm_r = np.asarray(inputs["csm_r"], np.float32)
    csm_i = np.asarray(inputs["csm_i"], np.float32)
    motions = np.asarray(inputs["motions"], np.float32)

    plans = [_plan_frame(traj[:, :, t], dcf[:, t]) for t in range(NT)]
    entries = _build_schedule(plans)

    ks16 = np.empty((M, 16), np.float32)
    ks16[:, 0::2] = kspace_r.T
    ks16[:, 1::2] = kspace_i.T

    W = _dft_matrix()
    import ml_dtypes
    wcomp = {"re": W.real, "im": W.imag, "imn": -W.imag}
    # packed W blocks: 15 w1 ([128u,320] per (comp,uc)) + 15 w2 unit tiles
    wall = np.zeros((128, NW, NX), np.float32)
    b = 0
    for comp in ("re", "im", "imn"):
        for uc in range(UC):
            wall[:, b] = wcomp[comp][uc * 128:(uc + 1) * 128]
            b += 1
    for comp in ("re", "im", "imn"):
        for k in range(5):
            wall[:, b] = wcomp[comp][UNIT_J[k]]
            b += 1
    wall = wall.astype(ml_dtypes.bfloat16)

    # csm pre-transposed for per-coil DMA: [NC, p, q, comp, x] where
    # y = q*128 + p (y >= NX zero-padded)
    csmT = np.zeros((NC, 2, 384, NX), np.float32)
    csmT[:, 0, :NX] = np.transpose(csm_r, (0, 2, 1))
    csmT[:, 1, :NX] = np.transpose(csm_i, (0, 2, 1))
    csmP = np.ascontiguousarray(
        csmT.reshape(NC, 2, 3, 128, NX).transpose(0, 3, 2, 1, 4)).astype(
        ml_dtypes.bfloat16)

    key = tuple(entries)
    if key not in _PROGRAM_CACHE:
        _PROGRAM_CACHE[key] = _build_program(entries)
    nc = _PROGRAM_CACHE[key]

    in_maps = []
    for t in range(NT):
        idx16, stream = _pack_frame(plans[t], entries, ks16)
        in_maps.append(dict(
            idx=idx16, stream=stream, wall=wall, csmP=csmP,
        ))

    res = run_bass_kernel_spmd(nc, in_maps, core_ids=list(range(NT)))

    total = np.zeros((NX, NX), np.complex64)
    for t in range(NT):
        imT = res.results[t]["imT"]
        im = (imT[0].T + 1j * imT[1].T).astype(np.complex64)
        total += _bilinear_warp_np(im, motions[:, :, :, t])
    out = np.stack([total.real, total.imag], axis=-1).astype(np.float32)
    return out


def _bilinear_warp_np(im, flow):
    Nx, Ny = im.shape
    xs = np.arange(Nx, dtype=np.float32)[:, None] + flow[..., 0]
    ys = np.arange(Ny, dtype=np.float32)[None, :] + flow[..., 1]
    xs = np.clip(xs, 0.0, Nx - 1.0)
    ys = np.clip(ys, 0.0, Ny - 1.0)
    x0 = np.floor(xs).astype(np.int32)
    y0 = np.floor(ys).astype(np.int32)
    x1 = np.minimum(x0 + 1, Nx - 1)
    y1 = np.minimum(y0 + 1, Ny - 1)
    dx = (xs - x0).astype(np.float32)
    dy = (ys - y0).astype(np.float32)
    return ((1 - dx) * (1 - dy) * im[x0, y0] + dx * (1 - dy) * im[x1, y0]
            + (1 - dx) * dy * im[x0, y1] + dx * dy * im[x1, y1])

